# revision 27
# baseline (speedup 1.0000x reference)
"""CLUB loss kernel for 8x TRN2 NeuronCores.

Math: per sample b (L=512 positions, D=64 dims):
  mu     = MLP_mu(x);  logvar = tanh(MLP_lv(x));  iv = exp(-logvar)
  loss = -0.5/(B*L) * sum_b X_b,
  X = sum_{d,l} (ysq - Ey2[d] - mu*2*(y - Ey[d])) * iv
    = finA - finBn + 2*dot(Eydup, smuiv) - civ
  with finA = sum ysq*iv, finBn = 2*sum mu*y*iv, smuiv = partials of mu*iv,
  civ = sum_d Ey2*sum_l iv.

Layout: every [64, L] elementwise tensor is (d, L-half)-stacked as [128, 256]
(partition p = d + 64*(l//256)), bf16 — halves the free dim every DVE/ACT
pass and merges the two L-halves into single ops. The cross-half moment sums
(Ey, Ey2 duplicated over both partition halves) come from one tiny bf16
matmul against an on-chip matrix M[p,q] = 1/512 iff p==q (mod 64), built by
two gpsimd affine_selects off the critical path. sy2 is mean-shifted by 256
before the bf16 rounding so Ey2 keeps fp32-grade accuracy; the shifted-out
constant reappears as a sum(siv) term in the final collapse. The C-term
folds as dot(Ey2delta_dup, siv) + sum(siv) since the dup vectors are
64-periodic.

Rounding consistency matters: sy2 and siv must sum the SAME rounded bf16
values that finA multiplies (ysq, iv) or the A-vs-civ cancellation picks up
a systematic ~2% bias — both use reduce_sum over the stored tensors.

All accumulator columns collapse on-chip in one fp32 matmul
(lhsT = [finA, finBn, smuiv, Ey2delta, ones], rhs = [ones, Eydup, siv]) so
the store is one single-packet [5,3] DMA.

Sharding: data-parallel over batch B=8, one sample per core; host combines.
"""

import os
import sys

if "/opt/trn_rl_repo" not in sys.path:
    sys.path.insert(0, "/opt/trn_rl_repo")

import numpy as np

B, L = 8, 512
XD, YD, H = 192, 64, 128
NCORES = 8
HC = L // 2
WIC = 640  # w1 a-parts (256) + b-parts (256) + w2 pack (128)

_CACHE: dict = {}

# Kernel-tail mode: "full" keeps Tile's drain + barriers + sem clear;
# "nobar" keeps only the drain (NRT's own postamble resets every semaphore
# and re-syncs the engines, so Tile's two all-engine barriers + RANGE_CLEAR
# are redundant); "nodrain" drops the drain too (output-DMA completion is
# then guaranteed by NRT's ring quiesce, not by the instruction stream).
TAIL = os.environ.get("CLUB_TAIL", "nodrain")


def _patch_tail(tc):
    if TAIL == "full":
        return
    import types

    def _drain_and_barrier(self, tick_clock, wait_clock):
        from concourse.vector_clock import ScopedClock

        popped = self.nc._tile_sem_poison_stack.pop()
        assert popped is self._sem_poison
        if TAIL == "nodrain":
            return
        nc = self.nc
        drain_inst = nc.sync.drain()
        wait_clock.add_sem_waits(
            drain_inst.ins, ScopedClock({None: tick_clock.global_clock})
        )
        # One-way broadcast instead of Tile's two full butterfly barriers:
        # no engine may enter NRT's per-engine postamble (which zeroes ALL
        # semaphores) until the drain has observed every kernel semaphore
        # settle — otherwise an early engine's reset block races a live wait.
        sem = nc.alloc_semaphore("tailbar")
        nc.sync.sem_inc(sem, 1)
        for eng_type, eng in nc.engines.items():
            if eng is nc.sync:
                continue
            eng.wait_ge(sem, 1)
        # SWDGE ring-metadata reset (normally part of clear_and_free) so a
        # second execution of the NEFF sees empty FIFOs.
        nc.gpsimd.dma_reset()

    tc._drain_and_barrier = types.MethodType(_drain_and_barrier, tc)


def build_nc(debug: bool = False):
    import concourse.bass as bass
    import concourse.bacc as bacc
    import concourse.tile as tile
    from concourse import mybir
    from concourse.tile import add_dep_helper

    f32 = mybir.dt.float32
    bf16 = mybir.dt.bfloat16
    AF = mybir.ActivationFunctionType
    OP = mybir.AluOpType
    AX = mybir.AxisListType

    nc = bacc.Bacc("TRN2", target_bir_lowering=False, debug=debug)

    # wiw2: w1 a-parts (cols 0:256) + w2 pack (cols 256:384)
    wi_d = nc.dram_tensor("wi", [128, 384], bf16, kind="ExternalInput")
    # xbw: xb (cols 0:512) + w1 b-parts (cols 512:768), all on partitions 64:128
    xb_d = nc.dram_tensor("xb", [64, 768], bf16, kind="ExternalInput")
    xa_d = nc.dram_tensor("xa", [128, L], bf16, kind="ExternalInput")
    yb_d = nc.dram_tensor("yb", [128, HC], bf16, kind="ExternalInput")
    bd_d = nc.dram_tensor("bd", [128, 4], f32, kind="ExternalInput")
    fin_d = nc.dram_tensor("fin", [5, 3], f32, kind="ExternalOutput")

    with tile.TileContext(nc) as tc:
        _patch_tail(tc)
        with (
            tc.tile_pool(name="sb", bufs=1) as sb,
            tc.tile_pool(name="ps", bufs=1, space=bass.MemorySpace.PSUM) as ps,
        ):
            # ---- input tiles ----
            wit = sb.tile([128, 384], bf16, tag="wit")
            xat = sb.tile([128, L], bf16, tag="xat")
            xbr = sb.tile([128, 768], bf16, tag="xbr")
            ybt = sb.tile([128, HC], bf16, tag="ybt")
            bdt = sb.tile([128, 4], f32, tag="bdt")

            # ---- accumulators / small constants ----
            # pacc cols: 0 sy, 1 sy2, 2 finA, 3 finBn, 4 smuiv, 5 Ey2delta, 6 ones
            pacc = sb.tile([128, 7], f32, tag="pacc")
            # rmat cols: 0 ones, 1 Eydup, 2 siv
            rmat = sb.tile([128, 3], f32, tag="rmat")
            nc.gpsimd.memset(rmat[:, 0:1], 1.0)
            nc.gpsimd.memset(pacc[:, 6:7], 1.0)

            # ---- input DMAs (one big contiguous transfer per ring) ----
            # scalar (ACT HWDGE ring): xb + w1 b-parts first, then a-parts + w2
            nc.scalar.dma_start(out=xbr[64:128, :], in_=xb_d[:, :])
            nc.scalar.dma_start(out=wit, in_=wi_d[:, :])
            # sync (SP HWDGE ring): xa, then biases
            nc.sync.dma_start(out=xat, in_=xa_d[:, :])
            nc.sync.dma_start(out=bdt, in_=bd_d[:, :])
            # gpsimd (SWDGE): y
            nc.gpsimd.dma_start(out=ybt, in_=yb_d[:, :])

            # ---- PE warm-up: the HAM clock gate keeps the PE at 1.2 GHz
            # until it has been busy ~3.4us; the PE is otherwise idle during
            # the input-DMA window, so dummy matmuls into h_lv[0] (overwritten
            # by the real start=True layer-1 matmul) flip it to 2.4 GHz before
            # the real work arrives. Garbage values never escape: alv0 zeroes
            # the bank via start=True.
            warm = sb.tile([128, 384], bf16, tag="warm")
            nc.gpsimd.memset(warm, 0.0)

            # ---- M matrix: M[p, q] = 1/512 iff p == q (mod 64), bf16 ----
            mones = sb.tile([128, 128], bf16, tag="mones")
            nc.gpsimd.memset(mones, 1.0 / 512.0)
            m1t = sb.tile([128, 128], bf16, tag="m1t")
            m2t = sb.tile([128, 128], bf16, tag="m2t")
            mft = sb.tile([128, 128], bf16, tag="mft")
            nc.gpsimd.affine_select(
                out=m1t, in_=mones, pattern=[[0, 2], [1, 64]],
                compare_op=OP.is_equal, fill=0.0, base=0, channel_multiplier=-1,
            )
            nc.gpsimd.affine_select(
                out=m2t, in_=mones, pattern=[[0, 2], [1, 64]],
                compare_op=OP.is_equal, fill=0.0, base=64, channel_multiplier=-1,
            )
            nc.gpsimd.tensor_tensor(out=mft, in0=m1t, in1=m2t, op=OP.add)

            # weight views
            w1lvT_a = wit[:, 0:128]
            w1muT_a = wit[:, 128:256]
            w1lvT_b = xbr[64:128, 512:640]
            w1muT_b = xbr[64:128, 640:768]
            w2lvT = wit[:, 256:320]
            w2muT = wit[:, 320:384]
            b1mu = bdt[:, 0:1]
            b1lv = bdt[:, 1:2]
            b2mu = bdt[:, 2:3]
            b2lv = bdt[:, 3:4]
            xb = xbr[64:128, 0:512]

            # ---- y moments (DVE, early; reduce_sum reads the STORED bf16
            # tensors so the sums match what finA later multiplies) ----
            ysq = sb.tile([128, HC], bf16, tag="ysq")
            dve_order = []
            dve_order.append(
                nc.vector.scalar_tensor_tensor(
                    out=ysq, in0=ybt, scalar=1.0, in1=ybt,
                    op0=OP.mult, op1=OP.mult,
                )
            )
            dve_order.append(
                nc.vector.reduce_sum(out=pacc[:, 0:1], in_=ybt, axis=AX.X)
            )
            dve_order.append(
                nc.vector.reduce_sum(out=pacc[:, 1:2], in_=ysq, axis=AX.X)
            )
            # bf16 inputs for the moment matmul: sy as-is, sy2 mean-shifted by
            # 256 so the bf16 rounding keeps ~fp32 absolute accuracy on Ey2.
            # gpsimd is idle here (it only issues DMAs + built M).
            mdup = sb.tile([128, 2], bf16, tag="mdup")
            nc.gpsimd.tensor_scalar(
                out=mdup[:, 0:1], in0=pacc[:, 0:1], scalar1=1.0, scalar2=None,
                op0=OP.mult,
            )
            nc.gpsimd.tensor_scalar(
                out=mdup[:, 1:2], in0=pacc[:, 1:2], scalar1=256.0, scalar2=None,
                op0=OP.subtract,
            )

            # ---- MLP layer 1: per-half PSUM tiles, 8 matmuls ----
            h_lv = [None, None]
            h_mu = [None, None]
            mm = {}
            h_lv[0] = ps.tile([128, HC], f32, tag="hlv0", name="hlv0")
            warm_mms = []
            for w in range(7):
                warm_mms.append(
                    nc.tensor.matmul(
                        h_lv[0], warm[:, 0:128], warm[:, 128:384],
                        start=True, stop=True, skip_group_check=True,
                    )
                )
            for c in range(2):
                cs = slice(c * HC, (c + 1) * HC)
                if c > 0:
                    h_lv[c] = ps.tile([128, HC], f32, tag=f"hlv{c}", name=f"hlv{c}")
                h_mu[c] = ps.tile([128, HC], f32, tag=f"hmu{c}", name=f"hmu{c}")
                mm[f"blv{c}"] = nc.tensor.matmul(
                    h_lv[c], w1lvT_b, xb[:, cs], start=True, stop=False
                )
                mm[f"alv{c}"] = nc.tensor.matmul(
                    h_lv[c], w1lvT_a, xat[:, cs], start=False, stop=True
                )
                mm[f"bmu{c}"] = nc.tensor.matmul(
                    h_mu[c], w1muT_b, xb[:, cs], start=True, stop=False
                )
                mm[f"amu{c}"] = nc.tensor.matmul(
                    h_mu[c], w1muT_a, xat[:, cs], start=False, stop=True
                )

            # relus: lv + mu-h0 on ACT, mu-h1 on DVE
            h_lv_s = sb.tile([128, L], bf16, tag="hlvs")
            h_mu_s = sb.tile([128, L], bf16, tag="hmus")
            act_order = []
            for c in range(2):
                cs = slice(c * HC, (c + 1) * HC)
                act_order.append(
                    nc.scalar.activation(
                        out=h_lv_s[:, cs], in_=h_lv[c], func=AF.Relu, bias=b1lv, scale=1.0
                    )
                )
            act_order.append(
                nc.scalar.activation(
                    out=h_mu_s[:, 0:HC], in_=h_mu[0], func=AF.Relu, bias=b1mu, scale=1.0
                )
            )
            dve_order.append(
                nc.vector.tensor_scalar(
                    out=h_mu_s[:, HC:L], in0=h_mu[1], scalar1=b1mu, scalar2=0.0,
                    op0=OP.add, op1=OP.max,
                )
            )

            # layer 2: (d, half)-stacked outputs
            lv_ps = ps.tile([128, HC], f32, tag="lvps")
            mu_ps = ps.tile([128, HC], f32, tag="mups")
            mm["w2lv0"] = nc.tensor.matmul(
                lv_ps[0:64, :], w2lvT, h_lv_s[:, 0:HC], start=True, stop=True
            )
            mm["w2lv1"] = nc.tensor.matmul(
                lv_ps[64:128, :], w2lvT, h_lv_s[:, HC:L], start=True, stop=True
            )
            mm["w2mu0"] = nc.tensor.matmul(
                mu_ps[0:64, :], w2muT, h_mu_s[:, 0:HC], start=True, stop=True
            )
            mm["w2mu1"] = nc.tensor.matmul(
                mu_ps[64:128, :], w2muT, h_mu_s[:, HC:L], start=True, stop=True
            )

            # ---- lv tail: tanh -> exp ----
            t1 = sb.tile([128, HC], bf16, tag="t1")
            iv = sb.tile([128, HC], bf16, tag="iv")
            act_order.append(
                nc.scalar.activation(out=t1, in_=lv_ps, func=AF.Tanh, bias=b2lv, scale=1.0)
            )
            act_order.append(
                nc.scalar.activation(out=iv, in_=t1, func=AF.Exp, scale=-1.0)
            )
            # siv: consistent sum of the stored bf16 iv; ACT is idle after exp
            scr3 = sb.tile([128, HC], bf16, tag="scr3")
            act_order.append(
                nc.scalar.activation(
                    out=scr3, in_=iv, func=AF.Copy, accum_out=rmat[:, 2:3]
                )
            )

            # ---- mu tail + reductions (DVE) ----
            m2 = sb.tile([128, HC], bf16, tag="m2")
            dve_order.append(
                nc.vector.scalar_tensor_tensor(
                    out=m2, in0=mu_ps, scalar=b2mu, in1=iv,
                    op0=OP.add, op1=OP.mult, accum_out=pacc[:, 4:5],
                )
            )
            scrA = sb.tile([128, HC], bf16, tag="scrA")
            dve_order.append(
                nc.vector.scalar_tensor_tensor(
                    out=scrA, in0=ysq, scalar=1.0, in1=iv,
                    op0=OP.mult, op1=OP.mult, accum_out=pacc[:, 2:3],
                )
            )
            scrB = sb.tile([128, HC], bf16, tag="scrB")
            dve_order.append(
                nc.vector.scalar_tensor_tensor(
                    out=scrB, in0=m2, scalar=2.0, in1=ybt,
                    op0=OP.mult, op1=OP.mult, accum_out=pacc[:, 3:4],
                )
            )

            # ---- moment duplication matmul (bf16, off the critical path) ----
            momdup_ps = ps.tile([128, 2], f32, tag="momps")
            mm["mom"] = nc.tensor.matmul(momdup_ps, mft, mdup, start=True, stop=True)
            act_order.append(nc.scalar.copy(out=rmat[:, 1:2], in_=momdup_ps[:, 0:1]))
            act_order.append(nc.scalar.copy(out=pacc[:, 5:6], in_=momdup_ps[:, 1:2]))

            # ---- collapse ----
            # lhsT = [finA, finBn, smuiv, Ey2delta, ones], rhs = [ones, Eydup, siv]
            fin_ps = ps.tile([5, 3], f32, tag="finps")
            mm["fin"] = nc.tensor.matmul(
                fin_ps, pacc[:, 2:7], rmat[:, 0:3], start=True, stop=True
            )
            fin_sb = sb.tile([5, 3], f32, tag="finsb")
            dve_order.append(nc.vector.tensor_copy(out=fin_sb, in_=fin_ps))
            nc.sync.dma_start(out=fin_d[:, :], in_=fin_sb, single_packet=True)

            # ---- stream ordering hints ----
            pe_order = warm_mms + [
                mm["blv0"], mm["blv1"], mm["bmu0"], mm["bmu1"],
                mm["alv0"], mm["alv1"], mm["amu0"], mm["amu1"],
                mm["w2lv0"], mm["w2lv1"], mm["w2mu0"], mm["w2mu1"],
                mm["mom"], mm["fin"],
            ]
            for order in (pe_order, act_order, dve_order):
                for a, b in zip(order[1:], order[:-1]):
                    add_dep_helper(a.ins, b.ins, sync=False, reason="stream-order")

    nc.compile()
    return nc


def pack_inputs(inputs: dict) -> list[dict]:
    import ml_dtypes

    bf = ml_dtypes.bfloat16
    x = np.asarray(inputs["x_samples"], dtype=np.float32)
    y = np.ascontiguousarray(np.asarray(inputs["y_samples"], dtype=np.float32))
    mu_W1 = np.asarray(inputs["mu_W1"], dtype=np.float32)
    mu_b1 = np.asarray(inputs["mu_b1"], dtype=np.float32)
    mu_W2 = np.asarray(inputs["mu_W2"], dtype=np.float32)
    mu_b2 = np.asarray(inputs["mu_b2"], dtype=np.float32)
    lv_W1 = np.asarray(inputs["lv_W1"], dtype=np.float32)
    lv_b1 = np.asarray(inputs["lv_b1"], dtype=np.float32)
    lv_W2 = np.asarray(inputs["lv_W2"], dtype=np.float32)
    lv_b2 = np.asarray(inputs["lv_b2"], dtype=np.float32)

    w1muT = mu_W1.T  # [192, 128]
    w1lvT = lv_W1.T
    wi = np.zeros((128, 384), bf)
    wi[:, 0:128] = w1lvT[0:128].astype(bf)
    wi[:, 128:256] = w1muT[0:128].astype(bf)
    wi[:, 256:320] = lv_W2.T.astype(bf)
    wi[:, 320:384] = mu_W2.T.astype(bf)

    bd = np.zeros((128, 4), np.float32)
    bd[:, 0] = mu_b1
    bd[:, 1] = lv_b1
    bd[:, 2] = np.concatenate([mu_b2, mu_b2])
    bd[:, 3] = np.concatenate([lv_b2, lv_b2])

    xb16 = x.astype(bf)
    xbw = np.zeros((NCORES, 64, 768), bf)
    for b in range(NCORES):
        xbw[b, :, 0:512] = xb16[b, 128:192]
        xbw[b, :, 512:640] = w1lvT[128:192].astype(bf)
        xbw[b, :, 640:768] = w1muT[128:192].astype(bf)
    in_maps = []
    for b in range(NCORES):
        yb = np.ascontiguousarray(
            y[b].reshape(64, 2, HC).transpose(1, 0, 2).reshape(128, HC)
        ).astype(bf)
        in_maps.append(
            {
                "wi": wi,
                "xa": np.ascontiguousarray(xb16[b, 0:128]),
                "xb": xbw[b],
                "yb": yb,
                "bd": bd,
            }
        )
    return in_maps


def _combine(results) -> float:
    tot = 0.0
    for r in results:
        f = r["fin"].astype(np.float64)  # [5, 3]
        # X = finA - finBn + 2*dot(smuiv, Eydup) - (dot(Ey2delta, siv) + sum(siv))
        tot += f[0, 0] - f[1, 0] + 2.0 * f[2, 1] - f[3, 2] - f[4, 2]
    return tot


def kernel(**inputs) -> np.ndarray:
    from concourse.bass_utils import run_bass_kernel_spmd

    if "nc" not in _CACHE:
        _CACHE["nc"] = build_nc(debug=False)
    nc = _CACHE["nc"]

    in_maps = pack_inputs(inputs)
    res = run_bass_kernel_spmd(nc, in_maps, core_ids=list(range(NCORES)))
    loss = -0.5 * _combine(res.results) / (B * L)
    return np.array(loss, dtype=np.float32)


# revision 28
# speedup vs baseline: 1.1128x; 1.1128x over previous
"""CLUB loss kernel for 8x TRN2 NeuronCores.

Math: per sample b (L=512 positions, D=64 dims):
  mu     = MLP_mu(x);  logvar = tanh(MLP_lv(x));  iv = exp(-logvar)
  loss = -0.5/(B*L) * sum_b X_b,
  X = sum_{d,l} (ysq - Ey2[d] - mu*2*(y - Ey[d])) * iv
    = finA - finBn + 2*dot(Eydup, smuiv) - civ
  with finA = sum ysq*iv, finBn = 2*sum mu*y*iv, smuiv = partials of mu*iv,
  civ = sum_d Ey2*sum_l iv.

Layout: every [64, L] elementwise tensor is (d, L-half)-stacked as [128, 256]
(partition p = d + 64*(l//256)), bf16 — halves the free dim every DVE/ACT
pass and merges the two L-halves into single ops. The cross-half moment sums
(Ey, Ey2 duplicated over both partition halves) come from one tiny bf16
matmul against an on-chip matrix M[p,q] = 1/512 iff p==q (mod 64), built by
two gpsimd affine_selects off the critical path. sy2 is mean-shifted by 256
before the bf16 rounding so Ey2 keeps fp32-grade accuracy; the shifted-out
constant reappears as a sum(siv) term in the final collapse. The C-term
folds as dot(Ey2delta_dup, siv) + sum(siv) since the dup vectors are
64-periodic.

Rounding consistency matters: sy2 and siv must sum the SAME rounded bf16
values that finA multiplies (ysq, iv) or the A-vs-civ cancellation picks up
a systematic ~2% bias — both use reduce_sum over the stored tensors.

All accumulator columns collapse on-chip in one fp32 matmul
(lhsT = [finA, finBn, smuiv, Ey2delta, ones], rhs = [ones, Eydup, siv]) so
the store is one single-packet [5,3] DMA.

Sharding: data-parallel over batch B=8, one sample per core; host combines.
"""

import os
import sys

if "/opt/trn_rl_repo" not in sys.path:
    sys.path.insert(0, "/opt/trn_rl_repo")

import numpy as np

B, L = 8, 512
XD, YD, H = 192, 64, 128
NCORES = 8
HC = L // 2
WIC = 640  # w1 a-parts (256) + b-parts (256) + w2 pack (128)

_CACHE: dict = {}

# Kernel-tail mode: "full" keeps Tile's drain + barriers + sem clear;
# "nobar" keeps only the drain (NRT's own postamble resets every semaphore
# and re-syncs the engines, so Tile's two all-engine barriers + RANGE_CLEAR
# are redundant); "nodrain" drops the drain too (output-DMA completion is
# then guaranteed by NRT's ring quiesce, not by the instruction stream).
TAIL = os.environ.get("CLUB_TAIL", "nodrain")


def _patch_tail(tc):
    if TAIL == "full":
        return
    import types

    def _drain_and_barrier(self, tick_clock, wait_clock):
        from concourse.vector_clock import ScopedClock

        popped = self.nc._tile_sem_poison_stack.pop()
        assert popped is self._sem_poison
        if TAIL == "nodrain":
            return
        nc = self.nc
        drain_inst = nc.sync.drain()
        wait_clock.add_sem_waits(
            drain_inst.ins, ScopedClock({None: tick_clock.global_clock})
        )
        # One-way broadcast instead of Tile's two full butterfly barriers:
        # no engine may enter NRT's per-engine postamble (which zeroes ALL
        # semaphores) until the drain has observed every kernel semaphore
        # settle — otherwise an early engine's reset block races a live wait.
        sem = nc.alloc_semaphore("tailbar")
        nc.sync.sem_inc(sem, 1)
        for eng_type, eng in nc.engines.items():
            if eng is nc.sync:
                continue
            eng.wait_ge(sem, 1)
        # SWDGE ring-metadata reset (normally part of clear_and_free) so a
        # second execution of the NEFF sees empty FIFOs.
        nc.gpsimd.dma_reset()

    tc._drain_and_barrier = types.MethodType(_drain_and_barrier, tc)


def build_nc(debug: bool = False):
    import concourse.bass as bass
    import concourse.bacc as bacc
    import concourse.tile as tile
    from concourse import mybir
    from concourse.tile import add_dep_helper

    f32 = mybir.dt.float32
    bf16 = mybir.dt.bfloat16
    AF = mybir.ActivationFunctionType
    OP = mybir.AluOpType
    AX = mybir.AxisListType

    nc = bacc.Bacc("TRN2", target_bir_lowering=False, debug=debug)

    # wiw2: w1 a-parts (cols 0:256) + w2 pack (cols 256:384)
    wi_d = nc.dram_tensor("wi", [128, 384], bf16, kind="ExternalInput")
    # xbw: xb (cols 0:512) + w1 b-parts (cols 512:768), all on partitions 64:128
    xb_d = nc.dram_tensor("xb", [64, 768], bf16, kind="ExternalInput")
    xa_d = nc.dram_tensor("xa", [128, L], bf16, kind="ExternalInput")
    yb_d = nc.dram_tensor("yb", [128, HC], bf16, kind="ExternalInput")
    bd_d = nc.dram_tensor("bd", [128, 4], f32, kind="ExternalInput")
    fin_d = nc.dram_tensor("fin", [5, 3], f32, kind="ExternalOutput")

    with tile.TileContext(nc) as tc:
        _patch_tail(tc)
        with (
            tc.tile_pool(name="sb", bufs=1) as sb,
            tc.tile_pool(name="ps", bufs=1, space=bass.MemorySpace.PSUM) as ps,
        ):
            # ---- input tiles ----
            wit = sb.tile([128, 384], bf16, tag="wit")
            xat = sb.tile([128, L], bf16, tag="xat")
            xbr = sb.tile([128, 768], bf16, tag="xbr")
            ybt = sb.tile([128, HC], bf16, tag="ybt")
            bdt = sb.tile([128, 4], f32, tag="bdt")

            # ---- accumulators / small constants ----
            # pacc cols: 0 sy, 1 sy2, 2 finA, 3 finBn, 4 smuiv, 5 Ey2delta, 6 ones
            pacc = sb.tile([128, 7], f32, tag="pacc")
            # rmat cols: 0 ones, 1 Eydup, 2 siv
            rmat = sb.tile([128, 3], f32, tag="rmat")
            nc.gpsimd.memset(rmat[:, 0:1], 1.0)
            nc.gpsimd.memset(pacc[:, 6:7], 1.0)

            # ---- input DMAs (one big contiguous transfer per ring) ----
            # scalar (ACT HWDGE ring): xb + w1 b-parts first, then a-parts + w2
            nc.scalar.dma_start(out=xbr[64:128, :], in_=xb_d[:, :])
            nc.scalar.dma_start(out=wit, in_=wi_d[:, :])
            # sync (SP HWDGE ring): xa, then biases
            nc.sync.dma_start(out=xat, in_=xa_d[:, :])
            nc.sync.dma_start(out=bdt, in_=bd_d[:, :])
            # gpsimd (SWDGE): y
            nc.gpsimd.dma_start(out=ybt, in_=yb_d[:, :])

            # ---- M matrix: M[p, q] = 1/512 iff p == q (mod 64), bf16 ----
            mones = sb.tile([128, 128], bf16, tag="mones")
            nc.gpsimd.memset(mones, 1.0 / 512.0)
            m1t = sb.tile([128, 128], bf16, tag="m1t")
            m2t = sb.tile([128, 128], bf16, tag="m2t")
            mft = sb.tile([128, 128], bf16, tag="mft")
            nc.gpsimd.affine_select(
                out=m1t, in_=mones, pattern=[[0, 2], [1, 64]],
                compare_op=OP.is_equal, fill=0.0, base=0, channel_multiplier=-1,
            )
            nc.gpsimd.affine_select(
                out=m2t, in_=mones, pattern=[[0, 2], [1, 64]],
                compare_op=OP.is_equal, fill=0.0, base=64, channel_multiplier=-1,
            )
            nc.gpsimd.tensor_tensor(out=mft, in0=m1t, in1=m2t, op=OP.add)

            # weight views
            w1lvT_a = wit[:, 0:128]
            w1muT_a = wit[:, 128:256]
            w1lvT_b = xbr[64:128, 512:640]
            w1muT_b = xbr[64:128, 640:768]
            w2lvT = wit[:, 256:320]
            w2muT = wit[:, 320:384]
            b1mu = bdt[:, 0:1]
            b1lv = bdt[:, 1:2]
            b2mu = bdt[:, 2:3]
            b2lv = bdt[:, 3:4]
            xb = xbr[64:128, 0:512]

            # ---- y moments (DVE, early; reduce_sum reads the STORED bf16
            # tensors so the sums match what finA later multiplies) ----
            ysq = sb.tile([128, HC], bf16, tag="ysq")
            dve_order = []
            dve_order.append(
                nc.vector.scalar_tensor_tensor(
                    out=ysq, in0=ybt, scalar=1.0, in1=ybt,
                    op0=OP.mult, op1=OP.mult,
                )
            )
            dve_order.append(
                nc.vector.reduce_sum(out=pacc[:, 0:1], in_=ybt, axis=AX.X)
            )
            dve_order.append(
                nc.vector.reduce_sum(out=pacc[:, 1:2], in_=ysq, axis=AX.X)
            )
            # bf16 inputs for the moment matmul: sy as-is, sy2 mean-shifted by
            # 256 so the bf16 rounding keeps ~fp32 absolute accuracy on Ey2.
            # gpsimd is idle here (it only issues DMAs + built M).
            mdup = sb.tile([128, 2], bf16, tag="mdup")
            nc.gpsimd.tensor_scalar(
                out=mdup[:, 0:1], in0=pacc[:, 0:1], scalar1=1.0, scalar2=None,
                op0=OP.mult,
            )
            nc.gpsimd.tensor_scalar(
                out=mdup[:, 1:2], in0=pacc[:, 1:2], scalar1=256.0, scalar2=None,
                op0=OP.subtract,
            )

            # ---- MLP layer 1: per-half PSUM tiles, 8 matmuls ----
            h_lv = [None, None]
            h_mu = [None, None]
            mm = {}
            for c in range(2):
                cs = slice(c * HC, (c + 1) * HC)
                h_lv[c] = ps.tile([128, HC], f32, tag=f"hlv{c}", name=f"hlv{c}")
                h_mu[c] = ps.tile([128, HC], f32, tag=f"hmu{c}", name=f"hmu{c}")
                mm[f"blv{c}"] = nc.tensor.matmul(
                    h_lv[c], w1lvT_b, xb[:, cs], start=True, stop=False
                )
                mm[f"alv{c}"] = nc.tensor.matmul(
                    h_lv[c], w1lvT_a, xat[:, cs], start=False, stop=True
                )
                mm[f"bmu{c}"] = nc.tensor.matmul(
                    h_mu[c], w1muT_b, xb[:, cs], start=True, stop=False
                )
                mm[f"amu{c}"] = nc.tensor.matmul(
                    h_mu[c], w1muT_a, xat[:, cs], start=False, stop=True
                )

            # relus: lv + mu-h0 on ACT, mu-h1 on DVE
            h_lv_s = sb.tile([128, L], bf16, tag="hlvs")
            h_mu_s = sb.tile([128, L], bf16, tag="hmus")
            act_order = []
            for c in range(2):
                cs = slice(c * HC, (c + 1) * HC)
                act_order.append(
                    nc.scalar.activation(
                        out=h_lv_s[:, cs], in_=h_lv[c], func=AF.Relu, bias=b1lv, scale=1.0
                    )
                )
            act_order.append(
                nc.scalar.activation(
                    out=h_mu_s[:, 0:HC], in_=h_mu[0], func=AF.Relu, bias=b1mu, scale=1.0
                )
            )
            dve_order.append(
                nc.vector.tensor_scalar(
                    out=h_mu_s[:, HC:L], in0=h_mu[1], scalar1=b1mu, scalar2=0.0,
                    op0=OP.add, op1=OP.max,
                )
            )

            # layer 2: (d, half)-stacked outputs
            lv_ps = ps.tile([128, HC], f32, tag="lvps")
            mu_ps = ps.tile([128, HC], f32, tag="mups")
            mm["w2lv0"] = nc.tensor.matmul(
                lv_ps[0:64, :], w2lvT, h_lv_s[:, 0:HC], start=True, stop=True
            )
            mm["w2lv1"] = nc.tensor.matmul(
                lv_ps[64:128, :], w2lvT, h_lv_s[:, HC:L], start=True, stop=True
            )
            mm["w2mu0"] = nc.tensor.matmul(
                mu_ps[0:64, :], w2muT, h_mu_s[:, 0:HC], start=True, stop=True
            )
            mm["w2mu1"] = nc.tensor.matmul(
                mu_ps[64:128, :], w2muT, h_mu_s[:, HC:L], start=True, stop=True
            )

            # ---- lv tail: tanh -> exp ----
            t1 = sb.tile([128, HC], bf16, tag="t1")
            iv = sb.tile([128, HC], bf16, tag="iv")
            act_order.append(
                nc.scalar.activation(out=t1, in_=lv_ps, func=AF.Tanh, bias=b2lv, scale=1.0)
            )
            act_order.append(
                nc.scalar.activation(out=iv, in_=t1, func=AF.Exp, scale=-1.0)
            )
            # siv: consistent sum of the stored bf16 iv; ACT is idle after exp
            scr3 = sb.tile([128, HC], bf16, tag="scr3")
            act_order.append(
                nc.scalar.activation(
                    out=scr3, in_=iv, func=AF.Copy, accum_out=rmat[:, 2:3]
                )
            )

            # ---- mu tail + reductions (DVE) ----
            m2 = sb.tile([128, HC], bf16, tag="m2")
            dve_order.append(
                nc.vector.scalar_tensor_tensor(
                    out=m2, in0=mu_ps, scalar=b2mu, in1=iv,
                    op0=OP.add, op1=OP.mult, accum_out=pacc[:, 4:5],
                )
            )
            scrA = sb.tile([128, HC], bf16, tag="scrA")
            dve_order.append(
                nc.vector.scalar_tensor_tensor(
                    out=scrA, in0=ysq, scalar=1.0, in1=iv,
                    op0=OP.mult, op1=OP.mult, accum_out=pacc[:, 2:3],
                )
            )
            scrB = sb.tile([128, HC], bf16, tag="scrB")
            dve_order.append(
                nc.vector.scalar_tensor_tensor(
                    out=scrB, in0=m2, scalar=2.0, in1=ybt,
                    op0=OP.mult, op1=OP.mult, accum_out=pacc[:, 3:4],
                )
            )

            # ---- moment duplication matmul (bf16, off the critical path) ----
            momdup_ps = ps.tile([128, 2], f32, tag="momps")
            mm["mom"] = nc.tensor.matmul(momdup_ps, mft, mdup, start=True, stop=True)
            act_order.append(nc.scalar.copy(out=rmat[:, 1:2], in_=momdup_ps[:, 0:1]))
            act_order.append(nc.scalar.copy(out=pacc[:, 5:6], in_=momdup_ps[:, 1:2]))

            # ---- collapse ----
            # lhsT = [finA, finBn, smuiv, Ey2delta, ones], rhs = [ones, Eydup, siv]
            fin_ps = ps.tile([5, 3], f32, tag="finps")
            mm["fin"] = nc.tensor.matmul(
                fin_ps, pacc[:, 2:7], rmat[:, 0:3], start=True, stop=True
            )
            fin_sb = sb.tile([5, 3], f32, tag="finsb")
            dve_order.append(nc.vector.tensor_copy(out=fin_sb, in_=fin_ps))
            nc.sync.dma_start(out=fin_d[:, :], in_=fin_sb, single_packet=True)

            # ---- stream ordering hints ----
            pe_order = [
                mm["blv0"], mm["blv1"], mm["bmu0"], mm["bmu1"],
                mm["alv0"], mm["alv1"], mm["amu0"], mm["amu1"],
                mm["w2lv0"], mm["w2lv1"], mm["w2mu0"], mm["w2mu1"],
                mm["mom"], mm["fin"],
            ]
            for order in (pe_order, act_order, dve_order):
                for a, b in zip(order[1:], order[:-1]):
                    add_dep_helper(a.ins, b.ins, sync=False, reason="stream-order")

    nc.compile()
    return nc


def pack_inputs(inputs: dict) -> list[dict]:
    import ml_dtypes

    bf = ml_dtypes.bfloat16
    x = np.asarray(inputs["x_samples"], dtype=np.float32)
    y = np.ascontiguousarray(np.asarray(inputs["y_samples"], dtype=np.float32))
    mu_W1 = np.asarray(inputs["mu_W1"], dtype=np.float32)
    mu_b1 = np.asarray(inputs["mu_b1"], dtype=np.float32)
    mu_W2 = np.asarray(inputs["mu_W2"], dtype=np.float32)
    mu_b2 = np.asarray(inputs["mu_b2"], dtype=np.float32)
    lv_W1 = np.asarray(inputs["lv_W1"], dtype=np.float32)
    lv_b1 = np.asarray(inputs["lv_b1"], dtype=np.float32)
    lv_W2 = np.asarray(inputs["lv_W2"], dtype=np.float32)
    lv_b2 = np.asarray(inputs["lv_b2"], dtype=np.float32)

    w1muT = mu_W1.T  # [192, 128]
    w1lvT = lv_W1.T
    wi = np.zeros((128, 384), bf)
    wi[:, 0:128] = w1lvT[0:128].astype(bf)
    wi[:, 128:256] = w1muT[0:128].astype(bf)
    wi[:, 256:320] = lv_W2.T.astype(bf)
    wi[:, 320:384] = mu_W2.T.astype(bf)

    bd = np.zeros((128, 4), np.float32)
    bd[:, 0] = mu_b1
    bd[:, 1] = lv_b1
    bd[:, 2] = np.concatenate([mu_b2, mu_b2])
    bd[:, 3] = np.concatenate([lv_b2, lv_b2])

    xb16 = x.astype(bf)
    xbw = np.zeros((NCORES, 64, 768), bf)
    for b in range(NCORES):
        xbw[b, :, 0:512] = xb16[b, 128:192]
        xbw[b, :, 512:640] = w1lvT[128:192].astype(bf)
        xbw[b, :, 640:768] = w1muT[128:192].astype(bf)
    in_maps = []
    for b in range(NCORES):
        yb = np.ascontiguousarray(
            y[b].reshape(64, 2, HC).transpose(1, 0, 2).reshape(128, HC)
        ).astype(bf)
        in_maps.append(
            {
                "wi": wi,
                "xa": np.ascontiguousarray(xb16[b, 0:128]),
                "xb": xbw[b],
                "yb": yb,
                "bd": bd,
            }
        )
    return in_maps


def _combine(results) -> float:
    tot = 0.0
    for r in results:
        f = r["fin"].astype(np.float64)  # [5, 3]
        # X = finA - finBn + 2*dot(smuiv, Eydup) - (dot(Ey2delta, siv) + sum(siv))
        tot += f[0, 0] - f[1, 0] + 2.0 * f[2, 1] - f[3, 2] - f[4, 2]
    return tot


def kernel(**inputs) -> np.ndarray:
    from concourse.bass_utils import run_bass_kernel_spmd

    if "nc" not in _CACHE:
        _CACHE["nc"] = build_nc(debug=False)
    nc = _CACHE["nc"]

    in_maps = pack_inputs(inputs)
    res = run_bass_kernel_spmd(nc, in_maps, core_ids=list(range(NCORES)))
    loss = -0.5 * _combine(res.results) / (B * L)
    return np.array(loss, dtype=np.float32)
